# revision 1
# baseline (speedup 1.0000x reference)
"""DoRA linear layer on 8 TRN2 NeuronCores.

out = (magnitude / ||W + s*B@A||_row) * (x @ (W + s*B@A)^T),  s = alpha/rank = 2.

Identity used: the reference's
    dora_out + base_out = mag_norm_scale * (base_out + s * lora_out)
                        = scale_o * (x @ W_adapted^T)

Sharding: TENSOR-PARALLEL on out_dim (per the sharding hint): core k owns
output columns [512k, 512(k+1)), x is replicated (streamed), W/lora_b/
magnitude are column-sharded.  This makes the norm/scale computation fully
LOCAL to each core — no cross-core exchange of any kind.

On-device, each core materializes its adapted weight column ONCE:
    W_ad^T = W^T + A^T @ B2^T      (32 K=16 matmuls + 32 DVE adds, fp16)
after which
  * the main GEMM is 64 token-tiles x ONE PSUM chain of 32 fp16 matmuls
    (no separate rank-16 path, no x@A^T precompute), and
  * the row norm is simply rowsum(W_ad^2): 32 DVE squares + 32 ones-matmul
    accumulations into a [1,512] PSUM, consistent to the bit with the
    weights the GEMM consumes.
scale = mag / sqrt(nsq) broadcast once into a [128,512] tile; every PSUM
drain is a single fused tensor_mul.

Host side only reshapes/transposes (layout prep), casts fp32 -> fp16
(accuracy budget is rel_err < 2e-2; fp16 gives ~4e-4), and concatenates
the per-core output column blocks.
"""

import sys

sys.path.insert(0, "/opt/trn_rl_repo")

import numpy as np

import concourse.bass as bass  # noqa: F401  (import keeps bass registered)
from concourse import bacc
import concourse.mybir as mybir
from concourse.tile import TileContext
from concourse.bass_utils import run_bass_kernel_spmd

FP32 = mybir.dt.float32
FP16 = mybir.dt.float16

NCORES = 8
TOK = 8192          # 4 * 2048 tokens total, all processed by every core
DIN = 4096
DOUT = 4096
RANK = 16
SCALING = 32.0 / 16

NI = DIN // 128     # 32 contraction blocks
OC = DOUT // NCORES  # 512 output columns per core
NB = TOK // 128     # 64 token tiles per core


def _build_program():
    nc = bacc.Bacc("TRN2", target_bir_lowering=False, debug=False,
                   num_devices=NCORES)

    # x in token-block-major layout: block t -> [128 part, NI*128] contiguous
    xb_d = nc.dram_tensor("xb", [NB, 128, NI * 128], FP16,
                          kind="ExternalInput")
    wt_d = nc.dram_tensor("wt", [NI, 128, OC], FP16, kind="ExternalInput")
    atr_d = nc.dram_tensor("atr", [RANK, NI, 128], FP16, kind="ExternalInput")
    b2n_d = nc.dram_tensor("b2n", [RANK, OC], FP16, kind="ExternalInput")
    magn_d = nc.dram_tensor("magn", [1, OC], FP32, kind="ExternalInput")
    out_d = nc.dram_tensor("out", [TOK, OC], FP32, kind="ExternalOutput")
    srow_d = nc.dram_tensor("srow_scratch", [1, OC], FP32)

    with TileContext(nc) as tc:
        with (
            tc.tile_pool(name="const", bufs=1) as const,
            tc.tile_pool(name="xbp", bufs=8) as xbp,
            tc.tile_pool(name="wp", bufs=32) as wp,
            tc.tile_pool(name="wadp", bufs=32) as wadp,
            tc.tile_pool(name="wsqp", bufs=4) as wsqp,
            tc.tile_pool(name="outp", bufs=10) as outp,
            tc.tile_pool(name="mp", bufs=6, space="PSUM") as mp,
            tc.tile_pool(name="sp", bufs=2, space="PSUM") as sp,
        ):
            atr = const.tile([RANK, NI, 128], FP16)
            nc.sync.dma_start(atr[:], atr_d[:])
            b2n_sb = const.tile([RANK, OC], FP16)
            nc.sync.dma_start(b2n_sb[:], b2n_d[:])
            magn_sb = const.tile([1, OC], FP32)
            nc.sync.dma_start(magn_sb[:], magn_d[:])
            ones128 = const.tile([128, 1], FP16)
            nc.vector.memset(ones128[:], 1.0)

            # --- W_ad^T = W^T + A^T @ B2^T, per 128-row i-block ----------
            # also accumulate nsq = colsum(W_ad^2) as the blocks appear.
            # W DMAs are front-loaded so the aux matmuls run back-to-back
            # (DMA-paced micro-gaps would pin the PE at a degraded p-state)
            w_ts = []
            for ib in range(NI):
                w_t = wp.tile([128, OC], FP16, tag="w", name=f"w{ib}")
                nc.sync.dma_start(w_t[:], wt_d[ib])
                w_ts.append(w_t)
            wads = []
            ps_nsq = sp.tile([1, OC], FP32, tag="nsq", name="psnsq")
            for ib in range(NI):
                w_t = w_ts[ib]
                ps_l = mp.tile([128, OC], FP32, tag="mp", name=f"pl{ib}")
                nc.tensor.matmul(ps_l[:], atr[:, ib, :], b2n_sb[:],
                                 start=True, stop=True)
                wad = wadp.tile([128, OC], FP16, tag="wad", name=f"wad{ib}")
                nc.vector.tensor_add(wad[:], ps_l[:], w_t[:])
                wads.append(wad)
                wsq = wsqp.tile([128, OC], FP16, tag="wsq", name=f"wsq{ib}")
                nc.vector.tensor_mul(wsq[:], wad[:], wad[:])
                nc.tensor.matmul(ps_nsq[:], ones128[:], wsq[:],
                                 start=(ib == 0), stop=(ib == NI - 1))

            # --- scale = mag / sqrt(nsq), broadcast over partitions ------
            nrmrow = const.tile([1, OC], FP32)
            srow = const.tile([1, OC], FP32)
            nc.scalar.activation(nrmrow[:], ps_nsq[:],
                                 mybir.ActivationFunctionType.Sqrt)
            nc.vector.reciprocal(nrmrow[:], nrmrow[:])
            nc.vector.tensor_mul(srow[:], nrmrow[:], magn_sb[:])
            sbc = const.tile([128, OC], FP32)
            nc.gpsimd.dma_start(srow_d[:], srow[:])
            _sl = srow_d[:]
            srow_bcast = bass.AP(
                tensor=_sl.tensor, offset=_sl.offset,
                ap=[[0, 128], [1, OC]])
            nc.gpsimd.dma_start(sbc[:], srow_bcast)

            # --- main GEMM: 64 token tiles, one PSUM chain each ----------
            for t in range(NB):
                xb = xbp.tile([128, NI * 128], FP16, tag="xb", name=f"xb{t}")
                nc.sync.dma_start(xb[:], xb_d[t])
                ps_m = mp.tile([128, OC], FP32, tag="mp", name=f"pm{t}")
                for ib in range(NI):
                    nc.tensor.matmul(
                        ps_m[:], xb[:, ib * 128:(ib + 1) * 128],
                        wads[ib][:], start=(ib == 0), stop=(ib == NI - 1))
                o_t = outp.tile([128, OC], FP32, tag="o", name=f"o{t}")
                nc.vector.tensor_mul(o_t[:], ps_m[:], sbc[:])
                nc.sync.dma_start(
                    out_d[t * 128:(t + 1) * 128, :], o_t[:])

    nc.compile()
    return nc


_PROGRAM = None


def _get_program():
    global _PROGRAM
    if _PROGRAM is None:
        _PROGRAM = _build_program()
    return _PROGRAM


def _prep_inputs(x, weight, lora_a_w, lora_b_w, magnitude):
    xr = np.asarray(x, dtype=np.float32).reshape(TOK, DIN)
    wr = np.asarray(weight, dtype=np.float32)
    ar = np.asarray(lora_a_w, dtype=np.float32)
    b2 = SCALING * np.asarray(lora_b_w, dtype=np.float32)

    # x token-block-major: [NB, 128 part(i%128), NI*128] per token block
    xT = xr.T.astype(np.float16)                       # [in, tok]
    xb = np.ascontiguousarray(
        xT.reshape(NI, 128, NB, 128).transpose(2, 1, 0, 3)
        .reshape(NB, 128, NI * 128))

    wT = wr.T.astype(np.float16)                       # [in, out]
    atr = np.ascontiguousarray(ar.astype(np.float16).reshape(RANK, NI, 128))
    b2t = b2.T.astype(np.float16)                      # [rank, out]
    mag32 = magnitude.astype(np.float32).reshape(1, DOUT)

    in_maps = []
    for cpu in range(NCORES):
        cs = slice(cpu * OC, (cpu + 1) * OC)
        wt = np.ascontiguousarray(wT[:, cs].reshape(NI, 128, OC))
        in_maps.append({
            "xb": xb, "wt": wt, "atr": atr,
            "b2n": np.ascontiguousarray(b2t[:, cs]),
            "magn": np.ascontiguousarray(mag32[:, cs]),
        })
    return in_maps


def kernel(x, weight, lora_a_w, lora_b_w, magnitude, _trace=False, **_kw):
    nc = _get_program()
    in_maps = _prep_inputs(x, weight, lora_a_w, lora_b_w, magnitude)
    res = run_bass_kernel_spmd(nc, in_maps, list(range(NCORES)), trace=_trace)
    out = np.concatenate([res.results[c]["out"] for c in range(NCORES)],
                         axis=1)
    if _trace:
        kernel._last_results = res
    return out.reshape(4, 2048, DOUT)



# revision 3
# speedup vs baseline: 1.0050x; 1.0050x over previous
"""DoRA linear layer on 8 TRN2 NeuronCores.

out = (magnitude / ||W + s*B@A||_row) * (x @ (W + s*B@A)^T),  s = alpha/rank = 2.

Identity used: the reference's
    dora_out + base_out = mag_norm_scale * (base_out + s * lora_out)
                        = scale_o * (x @ W_adapted^T)

Sharding: TENSOR-PARALLEL on out_dim: core k owns output columns
[512k, 512(k+1)), x replicated (streamed), W/lora_b/magnitude column-sharded.
Norm/scale computation is fully LOCAL to each core.

Schedule (the PE executes its queue in FIFO order, so emission order IS the
schedule):
  * W^T ships partition-major ([128, NI*OC] lines -> 8KiB DMA packets in 4
    chunks) instead of 32 x [128, OC] blocks (1KiB packets) -- the baseline's
    W DMA trickled until t=31.7us and starved the whole aux phase.
  * rank-16 LoRA matmuls are 4-way ROW-packed (tile_position=(32j,0), K=16)
    and the nsq ones-matmuls 4-way COLUMN-packed (tile_position=(0,32j), M=1)
    so the aux PE work drops from ~13.8us to ~3.6us.
  * aux rounds (4 blocks each) are interleaved with the first 3 token tiles'
    GEMM chains so the PE is never idle while the W_ad frontier advances
    (paced by the 32 serial DVE adds at ~620ns each).
  * W_ad squares run on the SCALAR engine (activation Square), keeping the
    DVE free for the adds.
  * scale = mag/sqrt(nsq) is broadcast to [128, OC] with a K=1 ones matmul
    (no DRAM round trip); the 4 nsq strips are summed via partition-aligned
    PSUM->SBUF copies + one ones-matmul (DVE cannot cross partitions).
  * PSUM budget 8 banks: 4 lora ring + 3 main ring + 1 nsq/scale; after the
    aux phase tiles t=3..6 reuse the freed lora banks, giving the main loop
    ~27us of drain slack so the scale path is never on the critical path.
  * inputs DMA on the sync queue, output stores on the scalar queue.

Host side only reshapes/transposes (layout prep), casts fp32 -> fp16
(accuracy budget is rel_err < 2e-2; fp16 gives ~4e-4), and concatenates
the per-core output column blocks.
"""

import sys

sys.path.insert(0, "/opt/trn_rl_repo")

import numpy as np

import concourse.bass as bass  # noqa: F401  (import keeps bass registered)
from concourse import bacc
import concourse.mybir as mybir
from concourse.tile import TileContext
from concourse.bass_utils import run_bass_kernel_spmd

FP32 = mybir.dt.float32
FP16 = mybir.dt.float16

NCORES = 8
TOK = 8192          # 4 * 2048 tokens total, all processed by every core
DIN = 4096
DOUT = 4096
RANK = 16
SCALING = 32.0 / 16

NI = DIN // 128      # 32 contraction blocks
OC = DOUT // NCORES  # 512 output columns per core
NB = TOK // 128      # 64 token tiles per core
NCH = 4              # W DMA chunks
CB = NI // NCH       # 8 blocks per chunk
NR = NI // 4         # 8 aux rounds of 4 blocks
ATW = NR * 128       # atr4 width in aux const


def _build_program():
    nc = bacc.Bacc("TRN2", target_bir_lowering=False, debug=False,
                   num_devices=NCORES)

    # x in token-block-major layout: block t -> [128 part, NI*128] contiguous
    xb_d = nc.dram_tensor("xb", [NB, 128, NI * 128], FP16,
                          kind="ExternalInput")
    # W^T partition-major: wc[g, p, s*OC+o] = W^T[(g*CB+s)*128 + p, o]
    wc_d = nc.dram_tensor("wc", [NCH, 128, CB * OC], FP16,
                          kind="ExternalInput")
    # aux = atr4 (row-packed A^T) ++ b2n4 (row-packed s*B^T)
    aux_d = nc.dram_tensor("aux", [128, ATW + OC], FP16, kind="ExternalInput")
    magn_d = nc.dram_tensor("magn", [1, OC], FP32, kind="ExternalInput")
    out_d = nc.dram_tensor("out", [TOK, OC], FP32, kind="ExternalOutput")

    with TileContext(nc) as tc:
        with (
            tc.tile_pool(name="const", bufs=1) as const,
            tc.tile_pool(name="xbp", bufs=8) as xbp,
            tc.tile_pool(name="wcp", bufs=NCH) as wcp,
            tc.tile_pool(name="wadp", bufs=NI) as wadp,
            tc.tile_pool(name="wsqp", bufs=4) as wsqp,
            tc.tile_pool(name="outp", bufs=10) as outp,
            tc.tile_pool(name="lorap", bufs=4, space="PSUM") as lorap,
            tc.tile_pool(name="mp", bufs=3, space="PSUM") as mp,
            tc.tile_pool(name="sp", bufs=1, space="PSUM") as sp,
        ):
            # ---- constants ------------------------------------------------
            aux = const.tile([128, ATW + OC], FP16)
            nc.sync.dma_start(aux[:], aux_d[:])
            magn_sb = const.tile([1, OC], FP32)
            nc.sync.dma_start(magn_sb[:], magn_d[:])
            ones128 = const.tile([128, 1], FP16)
            nc.vector.memset(ones128[:], 1.0)
            onesrow = const.tile([1, 128], FP16)
            nc.vector.memset(onesrow[:], 1.0)
            strip4 = const.tile([128, OC], FP16)
            nc.vector.memset(strip4[:], 0.0)

            # ---- input DMAs (sync queue, in priority order) ---------------
            wcs = []
            for g in range(NCH):
                w_c = wcp.tile([128, CB * OC], FP16, tag="wc", name=f"wc{g}")
                nc.sync.dma_start(w_c[:], wc_d[g])
                wcs.append(w_c)
            xb_tiles = {}
            for t in range(3):
                xb = xbp.tile([128, NI * 128], FP16, tag="xb", name=f"xb{t}")
                nc.sync.dma_start(xb[:], xb_d[t])
                xb_tiles[t] = xb

            # ---- aux + early-main rounds ----------------------------------
            # round r covers blocks ib = 4r+j.  LoRA matmuls for round r+1 are
            # emitted at the top of round r so the packed group is ready when
            # its ring slots free up (the wad adds of round r).
            ps_nsq = sp.tile([128, OC], FP32, tag="sp", name="psnsq")
            mains = [mp.tile([128, OC], FP32, tag="mp", name=f"pm{t}")
                     for t in range(3)]
            wads = [None] * NI
            lora_ps = {}

            def emit_lora_group(r):
                for j in range(4):
                    ib = 4 * r + j
                    pl = lorap.tile([128, OC], FP32, tag="pl",
                                    name=f"pl{ib}")
                    nc.tensor.matmul(
                        pl[:], aux[32 * j:32 * j + RANK, r * 128:(r + 1) * 128],
                        aux[32 * j:32 * j + RANK, ATW:ATW + OC],
                        start=True, stop=True, tile_position=(32 * j, 0))
                    lora_ps[ib] = pl

            emit_lora_group(0)
            for r in range(NR):
                if r + 1 < NR:
                    emit_lora_group(r + 1)
                for j in range(4):
                    ib = 4 * r + j
                    wad = wadp.tile([128, OC], FP16, tag="wad",
                                    name=f"wad{ib}")
                    nc.vector.tensor_add(
                        wad[:], lora_ps[ib][:],
                        wcs[ib // CB][:, (ib % CB) * OC:(ib % CB + 1) * OC])
                    wads[ib] = wad
                for j in range(4):
                    ib = 4 * r + j
                    wsq = wsqp.tile([128, OC], FP16, tag="wsq",
                                    name=f"wsq{ib}")
                    nc.scalar.square(wsq[:], wads[ib][:])
                    nc.tensor.matmul(
                        ps_nsq[32 * j:32 * j + 1, :], ones128[:], wsq[:],
                        start=(r == 0), stop=(r == NR - 1),
                        tile_position=(0, 32 * j))
                # early-main: 3 tiles advance to the current wad frontier
                for t in range(3):
                    for j in range(4):
                        ib = 4 * r + j
                        nc.tensor.matmul(
                            mains[t][:],
                            xb_tiles[t][:, ib * 128:(ib + 1) * 128],
                            wads[ib][:], start=(ib == 0), stop=(ib == NI - 1))

            # ---- scale = mag / sqrt(nsq), broadcast to [128, OC] ----------
            # DVE lanes cannot cross partitions: move the 4 strips into an
            # SBUF tile (partition-aligned copies), reduce them with a ones
            # matmul, then sqrt / recip / mul and broadcast with a K=1 matmul.
            for j in range(4):
                nc.vector.tensor_copy(strip4[32 * j:32 * j + 1, :],
                                      ps_nsq[32 * j:32 * j + 1, :])
            ps_red = sp.tile([128, OC], FP32, tag="sp", name="psred")
            nc.tensor.matmul(ps_red[0:1, :], ones128[:], strip4[:],
                             start=True, stop=True)
            nrmrow = const.tile([1, OC], FP32)
            nc.scalar.sqrt(nrmrow[:], ps_red[0:1, :])
            invrow = const.tile([1, OC], FP32)
            nc.vector.reciprocal_approx_fast(invrow[:], nrmrow[:])
            srow = const.tile([1, OC], FP16)
            nc.vector.tensor_mul(srow[:], invrow[:], magn_sb[:])
            ps_b = sp.tile([128, OC], FP32, tag="sp", name="psb")
            nc.tensor.matmul(ps_b[:], onesrow[:], srow[:],
                             start=True, stop=True)
            sbc = const.tile([128, OC], FP32)
            nc.vector.tensor_copy(sbc[:], ps_b[:])

            # ---- drains for the 3 early tiles (stores on scalar queue) ----
            for t in range(3):
                o_t = outp.tile([128, OC], FP32, tag="o", name=f"o{t}")
                nc.vector.tensor_mul(o_t[:], mains[t][:], sbc[:])
                nc.scalar.dma_start(out_d[t * 128:(t + 1) * 128, :], o_t[:])

            # ---- main GEMM: remaining 61 token tiles ----------------------
            # tiles 3..6 use the freed lora PSUM banks -> 7 banks cycling.
            for t in range(3, NB):
                xb = xbp.tile([128, NI * 128], FP16, tag="xb", name=f"xb{t}")
                nc.sync.dma_start(xb[:], xb_d[t])
                pool = lorap if t < 7 else mp
                tag = "pl" if t < 7 else "mp"
                ps_m = pool.tile([128, OC], FP32, tag=tag, name=f"pm{t}")
                for ib in range(NI):
                    nc.tensor.matmul(
                        ps_m[:], xb[:, ib * 128:(ib + 1) * 128],
                        wads[ib][:], start=(ib == 0), stop=(ib == NI - 1))
                if t < NB - 1:
                    o_t = outp.tile([128, OC], FP32, tag="o", name=f"o{t}")
                    nc.vector.tensor_mul(o_t[:], ps_m[:], sbc[:])
                    nc.scalar.dma_start(
                        out_d[t * 128:(t + 1) * 128, :], o_t[:])
                else:
                    # split the last drain so the first half's store overlaps
                    # the second half's multiply (shortens the tail)
                    for h in range(2):
                        oh = outp.tile([128, OC // 2], FP32, tag=f"oh{h}",
                                       name=f"oh{t}_{h}")
                        cs = slice(h * (OC // 2), (h + 1) * (OC // 2))
                        nc.vector.tensor_mul(oh[:], ps_m[:, cs], sbc[:, cs])
                        nc.scalar.dma_start(
                            out_d[t * 128:(t + 1) * 128, cs], oh[:])

    nc.compile()
    return nc


_PROGRAM = None


def _get_program():
    global _PROGRAM
    if _PROGRAM is None:
        _PROGRAM = _build_program()
    return _PROGRAM


def _prep_inputs(x, weight, lora_a_w, lora_b_w, magnitude):
    xr = np.asarray(x, dtype=np.float32).reshape(TOK, DIN)
    wr = np.asarray(weight, dtype=np.float32)
    ar = np.asarray(lora_a_w, dtype=np.float32)
    b2 = SCALING * np.asarray(lora_b_w, dtype=np.float32)

    # x token-block-major: [NB, 128 part(i%128), NI*128] per token block
    xT = xr.T.astype(np.float16)                       # [in, tok]
    xb = np.ascontiguousarray(
        xT.reshape(NI, 128, NB, 128).transpose(2, 1, 0, 3)
        .reshape(NB, 128, NI * 128))

    wT = wr.T.astype(np.float16)                       # [in, out]
    b2t = b2.T.astype(np.float16)                      # [rank, out]
    mag32 = magnitude.astype(np.float32).reshape(1, DOUT)

    # atr4: row-packed A^T.  atr4[32j + r, h*128 + c] = A[r, (4h+j)*128 + c]
    atr4 = np.zeros((4, 32, ATW), dtype=np.float16)
    Ar = ar.astype(np.float16).reshape(RANK, NR, 4, 128)   # [r, h, j, c]
    atr4[:, :RANK, :] = Ar.transpose(2, 0, 1, 3).reshape(4, RANK, ATW)
    atr4 = atr4.reshape(128, ATW)

    in_maps = []
    for cpu in range(NCORES):
        cs = slice(cpu * OC, (cpu + 1) * OC)
        # W^T partition-major in CB-block chunks
        wc = np.ascontiguousarray(
            wT[:, cs].reshape(NCH, CB, 128, OC).transpose(0, 2, 1, 3)
            .reshape(NCH, 128, CB * OC))
        # b2n4: row-packed s*B^T replicated into the 4 row strips
        b24 = np.zeros((4, 32, OC), dtype=np.float16)
        b24[:, :RANK, :] = b2t[None, :, cs]
        aux = np.concatenate([atr4, b24.reshape(128, OC)], axis=1)
        in_maps.append({
            "xb": xb, "wc": wc,
            "aux": np.ascontiguousarray(aux),
            "magn": np.ascontiguousarray(mag32[:, cs]),
        })
    return in_maps


def kernel(x, weight, lora_a_w, lora_b_w, magnitude, _trace=False, **_kw):
    nc = _get_program()
    in_maps = _prep_inputs(x, weight, lora_a_w, lora_b_w, magnitude)
    res = run_bass_kernel_spmd(nc, in_maps, list(range(NCORES)), trace=_trace)
    out = np.concatenate([res.results[c]["out"] for c in range(NCORES)],
                         axis=1)
    if _trace:
        kernel._last_results = res
    return out.reshape(4, 2048, DOUT)


# revision 7
# speedup vs baseline: 1.0333x; 1.0281x over previous
"""DoRA linear layer on 8 TRN2 NeuronCores.

out = (magnitude / ||W + s*B@A||_row) * (x @ (W + s*B@A)^T),  s = alpha/rank = 2.

Identity used: the reference's
    dora_out + base_out = mag_norm_scale * (base_out + s * lora_out)
                        = scale_o * (x @ W_adapted^T)

Sharding: TENSOR-PARALLEL on out_dim: core k owns output columns
[512k, 512(k+1)), x replicated (streamed), W/lora_b/magnitude column-sharded.
Norm/scale computation is fully LOCAL to each core.

Schedule notes (the PE executes its queue in FIFO order, so emission order IS
the schedule; measured constants from the NTFF profile):
  * ~8.7us fixed DMA dead time at kernel start, then ~420GB/s aggregate.
  * W^T ships partition-major (8KiB DMA lines, 4 chunks of 8 blocks) instead
    of 32 x [128, OC] blocks with 1KiB lines -- the baseline's W DMA trickled
    until t=31.7us and starved the aux phase.
  * The W_ad adds (PSUM fp32 + W fp16 -> fp16) are DVE-bound at ~677ns per
    [128,512] block (PSUM reads run 1x mode).  They are batched 2 blocks per
    op over two-bank lora PSUM tiles (~1.16us/pair) and form the aux
    critical chain; everything else pipelines around them:
      - rank-16 LoRA matmuls 4-way ROW-packed (tile_position=(32j,0), K=16),
      - nsq ones-matmuls 4-way COLUMN-packed (tile_position=(0,32j), M=1),
      - W_ad squares batched on the SCALAR engine (activation Square),
      - per aux round r the PE block [lora(r+1) | main(r) | nsq(r)] is
        emitted after the adds of round r, so adds for round r+1 always run
        one round ahead of the PE and the PE never head-of-line blocks.
  * Token tiles t0-t2 join the aux rounds adaptively (t0 at r0, t1 at r2,
    t2 at r4, with catch-up bursts) so the PE stays fed while xb tiles
    arrive; the sync-queue DMA order (aux, wc0, xb0, wc1, xb1, wc2, xb2,
    wc3, xb3..) matches that schedule.
  * scale = mag/sqrt(nsq) broadcasts to [128, OC] via a K=1 ones matmul (no
    DRAM round trip); the 4 nsq strips are reduced with partition-aligned
    PSUM->SBUF copies + one ones-matmul (DVE lanes cannot cross partitions).
  * PSUM budget 8 banks: 2x2 lora ring + 3 main ring + 1 nsq/scale; after
    the aux phase tiles t3-t6 run as half-chains in the freed lora tiles,
    giving ~27us of drain slack so the scale path never stalls the PE.
  * Input DMAs issue on the sync queue, output stores on the scalar queue;
    the last tile's drain is split in halves to shorten the tail.

Host side only reshapes/transposes (layout prep), casts fp32 -> fp16
(accuracy budget is rel_err < 2e-2; fp16 gives ~4e-4), and concatenates
the per-core output column blocks.
"""

import sys

sys.path.insert(0, "/opt/trn_rl_repo")

import numpy as np

import concourse.bass as bass  # noqa: F401  (import keeps bass registered)
from concourse import bacc
import concourse.mybir as mybir
from concourse.tile import TileContext
from concourse.bass_utils import run_bass_kernel_spmd

FP32 = mybir.dt.float32
FP16 = mybir.dt.float16

NCORES = 8
TOK = 8192          # 4 * 2048 tokens total, all processed by every core
DIN = 4096
DOUT = 4096
RANK = 16
SCALING = 32.0 / 16

NI = DIN // 128      # 32 contraction blocks
OC = DOUT // NCORES  # 512 output columns per core
NB = TOK // 128      # 64 token tiles per core
NCH = 4              # W DMA chunks
CB = NI // NCH       # 8 blocks per chunk
NR = NI // 4         # 8 aux rounds of 4 blocks
ATW = NR * 128       # atr4 width in aux const
JOIN = {0: 0, 1: 2, 2: 4}   # aux round at which early tile t joins


def _build_program():
    nc = bacc.Bacc("TRN2", target_bir_lowering=False, debug=False,
                   num_devices=NCORES)

    # x in token-block-major layout: block t -> [128 part, NI*128] contiguous
    xb_d = nc.dram_tensor("xb", [NB, 128, NI * 128], FP16,
                          kind="ExternalInput")
    # W^T partition-major: wc[g, p, s*OC+o] = W^T[(g*CB+s)*128 + p, o]
    wc_d = nc.dram_tensor("wc", [NCH, 128, CB * OC], FP16,
                          kind="ExternalInput")
    # aux = atr4 (row-packed A^T) ++ b2n4 (row-packed s*B^T)
    aux_d = nc.dram_tensor("aux", [128, ATW + OC], FP16, kind="ExternalInput")
    magn_d = nc.dram_tensor("magn", [1, OC], FP32, kind="ExternalInput")
    out_d = nc.dram_tensor("out", [TOK, OC], FP32, kind="ExternalOutput")

    with TileContext(nc) as tc:
        with (
            tc.tile_pool(name="const", bufs=1) as const,
            tc.tile_pool(name="xbp", bufs=8) as xbp,
            tc.tile_pool(name="wcp", bufs=NCH) as wcp,
            tc.tile_pool(name="wadp", bufs=NI // 2) as wadp,
            tc.tile_pool(name="wsqp", bufs=2) as wsqp,
            tc.tile_pool(name="outp", bufs=10) as outp,
            tc.tile_pool(name="lorap", bufs=2, space="PSUM") as lorap,
            tc.tile_pool(name="mp", bufs=3, space="PSUM") as mp,
            tc.tile_pool(name="sp", bufs=1, space="PSUM") as sp,
        ):
            # ---- constants ------------------------------------------------
            aux = const.tile([128, ATW + OC], FP16)
            nc.sync.dma_start(aux[:], aux_d[:])
            magn_sb = const.tile([1, OC], FP32)
            nc.sync.dma_start(magn_sb[:], magn_d[:])
            ones128 = const.tile([128, 1], FP16)
            nc.vector.memset(ones128[:], 1.0)
            onesrow = const.tile([1, 128], FP16)
            nc.vector.memset(onesrow[:], 1.0)
            strip4 = const.tile([128, OC], FP16)
            nc.vector.memset(strip4[:], 0.0)

            # ---- input DMAs (sync queue order == emission order) ----------
            wcs, xb_tiles = [], {}
            for g in range(NCH):
                w_c = wcp.tile([128, CB * OC], FP16, tag="wc", name=f"wc{g}")
                nc.sync.dma_start(w_c[:], wc_d[g])
                wcs.append(w_c)
                if g < 3:
                    xb = xbp.tile([128, NI * 128], FP16, tag="xb",
                                  name=f"xb{g}")
                    nc.sync.dma_start(xb[:], xb_d[g])
                    xb_tiles[g] = xb

            # ---- aux rounds interleaved with early-main ------------------
            ps_nsq = sp.tile([128, OC], FP32, tag="sp", name="psnsq")
            mains = [mp.tile([128, OC], FP32, tag="mp", name=f"pm{t}")
                     for t in range(3)]
            wad2 = [None] * (NI // 2)    # pair k covers blocks (2k, 2k+1)
            wsq2 = [None] * (NI // 2)
            lora_ps = {}                 # round r -> (tileA, tileB)

            def wad_ap(ib):
                return wad2[ib // 2][:, (ib % 2) * OC:(ib % 2 + 1) * OC]

            def emit_lora_group(r):
                plA = lorap.tile([128, 2 * OC], FP32, tag="pl",
                                 name=f"plA{r}")
                plB = lorap.tile([128, 2 * OC], FP32, tag="pl",
                                 name=f"plB{r}")
                for j in range(4):
                    dst = (plA if j < 2 else plB)
                    nc.tensor.matmul(
                        dst[:, (j % 2) * OC:(j % 2 + 1) * OC],
                        aux[32 * j:32 * j + RANK, r * 128:(r + 1) * 128],
                        aux[32 * j:32 * j + RANK, ATW:ATW + OC],
                        start=True, stop=True, tile_position=(32 * j, 0))
                lora_ps[r] = (plA, plB)

            def emit_main(t, ib):
                nc.tensor.matmul(
                    mains[t][:], xb_tiles[t][:, ib * 128:(ib + 1) * 128],
                    wad_ap(ib), start=(ib == 0), stop=(ib == NI - 1))

            emit_lora_group(0)
            for r in range(NR):
                # DVE adds for round r (2-block batched over the lora tiles)
                plA, plB = lora_ps[r]
                for half, pl in enumerate((plA, plB)):
                    k = 2 * r + half
                    w2 = wadp.tile([128, 2 * OC], FP16, tag="wad",
                                   name=f"wad{k}")
                    nc.vector.tensor_add(
                        w2[:], pl[:],
                        wcs[k // (CB // 2)][:, (k % (CB // 2)) * 2 * OC:
                                            (k % (CB // 2) + 1) * 2 * OC])
                    wad2[k] = w2
                    wsq = wsqp.tile([128, 2 * OC], FP16, tag="wsq",
                                    name=f"wsq{k}")
                    nc.scalar.square(wsq[:], w2[:])
                    wsq2[k] = wsq
                # PE block for round r (runs while adds of r+1 trickle)
                if r + 1 < NR:
                    emit_lora_group(r + 1)
                for t in range(3):
                    if r == JOIN[t]:
                        for ib in range(0, 4 * r + 4):
                            emit_main(t, ib)
                    elif r > JOIN[t]:
                        for ib in range(4 * r, 4 * r + 4):
                            emit_main(t, ib)
                for j in range(4):
                    k = 2 * r + j // 2
                    wsq = wsq2[k]
                    nc.tensor.matmul(
                        ps_nsq[32 * j:32 * j + 1, :],
                        ones128[:], wsq[:, (j % 2) * OC:(j % 2 + 1) * OC],
                        start=(r == 0), stop=(r == NR - 1),
                        tile_position=(0, 32 * j))

            # ---- scale = mag / sqrt(nsq), broadcast to [128, OC] ----------
            for j in range(4):
                nc.vector.tensor_copy(strip4[32 * j:32 * j + 1, :],
                                      ps_nsq[32 * j:32 * j + 1, :])
            ps_red = sp.tile([128, OC], FP32, tag="sp", name="psred")
            nc.tensor.matmul(ps_red[0:1, :], ones128[:], strip4[:],
                             start=True, stop=True)
            nrmrow = const.tile([1, OC], FP32)
            nc.scalar.sqrt(nrmrow[:], ps_red[0:1, :])
            invrow = const.tile([1, OC], FP32)
            nc.vector.reciprocal_approx_fast(invrow[:], nrmrow[:])
            srow = const.tile([1, OC], FP16)
            nc.vector.tensor_mul(srow[:], invrow[:], magn_sb[:])
            ps_b = sp.tile([128, OC], FP32, tag="sp", name="psb")
            nc.tensor.matmul(ps_b[:], onesrow[:], srow[:],
                             start=True, stop=True)
            sbc = const.tile([128, OC], FP32)
            nc.vector.tensor_copy(sbc[:], ps_b[:])

            def drain(ps_ap, t):
                if t < NB - 1:
                    o_t = outp.tile([128, OC], FP32, tag="o", name=f"o{t}")
                    nc.vector.tensor_mul(o_t[:], ps_ap, sbc[:])
                    nc.scalar.dma_start(
                        out_d[t * 128:(t + 1) * 128, :], o_t[:])
                else:
                    # split the last drain so the first half's store
                    # overlaps the second half's multiply (shorter tail)
                    for h in range(2):
                        oh = outp.tile([128, OC // 2], FP32, tag=f"oh{h}",
                                       name=f"oh{t}_{h}")
                        cs = slice(h * (OC // 2), (h + 1) * (OC // 2))
                        nc.vector.tensor_mul(oh[:], ps_ap[:, cs], sbc[:, cs])
                        nc.scalar.dma_start(
                            out_d[t * 128:(t + 1) * 128, cs], oh[:])

            for t in range(3):
                drain(mains[t][:], t)

            # ---- main GEMM: remaining 61 token tiles ----------------------
            # t3-t6 run as half-chains in the freed lora tiles (2 banks
            # each); t7.. cycle through the 3 mp banks.
            big = {}
            for t in range(3, NB):
                xb = xbp.tile([128, NI * 128], FP16, tag="xb", name=f"xb{t}")
                nc.sync.dma_start(xb[:], xb_d[t])
                xb_tiles[t] = xb
                if t < 7:
                    if t in (3, 5):
                        big[t] = lorap.tile([128, 2 * OC], FP32, tag="pl",
                                            name=f"pmL{t}")
                    base = big[t] if t in (3, 5) else big[t - 1]
                    ps_ap = base[:, (t % 2 == 0) * OC:
                                 ((t % 2 == 0) + 1) * OC]
                else:
                    ps_m = mp.tile([128, OC], FP32, tag="mp", name=f"pm{t}")
                    ps_ap = ps_m[:]
                for ib in range(NI):
                    nc.tensor.matmul(
                        ps_ap, xb[:, ib * 128:(ib + 1) * 128],
                        wad_ap(ib), start=(ib == 0), stop=(ib == NI - 1))
                drain(ps_ap, t)

    nc.compile()
    return nc


_PROGRAM = None


def _get_program():
    global _PROGRAM
    if _PROGRAM is None:
        _PROGRAM = _build_program()
    return _PROGRAM


def _prep_inputs(x, weight, lora_a_w, lora_b_w, magnitude):
    xr = np.asarray(x, dtype=np.float32).reshape(TOK, DIN)
    wr = np.asarray(weight, dtype=np.float32)
    ar = np.asarray(lora_a_w, dtype=np.float32)
    b2 = SCALING * np.asarray(lora_b_w, dtype=np.float32)

    # x token-block-major: [NB, 128 part(i%128), NI*128] per token block
    xT = xr.T.astype(np.float16)                       # [in, tok]
    xb = np.ascontiguousarray(
        xT.reshape(NI, 128, NB, 128).transpose(2, 1, 0, 3)
        .reshape(NB, 128, NI * 128))

    wT = wr.T.astype(np.float16)                       # [in, out]
    b2t = b2.T.astype(np.float16)                      # [rank, out]
    mag32 = magnitude.astype(np.float32).reshape(1, DOUT)

    # atr4: row-packed A^T.  atr4[32j + r, h*128 + c] = A[r, (4h+j)*128 + c]
    atr4 = np.zeros((4, 32, ATW), dtype=np.float16)
    Ar = ar.astype(np.float16).reshape(RANK, NR, 4, 128)   # [r, h, j, c]
    atr4[:, :RANK, :] = Ar.transpose(2, 0, 1, 3).reshape(4, RANK, ATW)
    atr4 = atr4.reshape(128, ATW)

    in_maps = []
    for cpu in range(NCORES):
        cs = slice(cpu * OC, (cpu + 1) * OC)
        # W^T partition-major in CB-block chunks
        wc = np.ascontiguousarray(
            wT[:, cs].reshape(NCH, CB, 128, OC).transpose(0, 2, 1, 3)
            .reshape(NCH, 128, CB * OC))
        # b2n4: row-packed s*B^T replicated into the 4 row strips
        b24 = np.zeros((4, 32, OC), dtype=np.float16)
        b24[:, :RANK, :] = b2t[None, :, cs]
        aux = np.concatenate([atr4, b24.reshape(128, OC)], axis=1)
        in_maps.append({
            "xb": xb, "wc": wc,
            "aux": np.ascontiguousarray(aux),
            "magn": np.ascontiguousarray(mag32[:, cs]),
        })
    return in_maps


def kernel(x, weight, lora_a_w, lora_b_w, magnitude, _trace=False, **_kw):
    nc = _get_program()
    in_maps = _prep_inputs(x, weight, lora_a_w, lora_b_w, magnitude)
    res = run_bass_kernel_spmd(nc, in_maps, list(range(NCORES)), trace=_trace)
    out = np.concatenate([res.results[c]["out"] for c in range(NCORES)],
                         axis=1)
    if _trace:
        kernel._last_results = res
    return out.reshape(4, 2048, DOUT)
